# revision 69
# baseline (speedup 1.0000x reference)
"""BiDAF attention-flow kernel for Trainium2 (8 NeuronCores, data-parallel over batch).

Per core (one batch element):
  s[j,i]   = c[j] + q[i] + sum_h w_cq[h]*emb2[j,h]*emb1[i,h]
  a        = softmax_i(s)          (c[j] drops out of the row softmax)
  y2x      = a @ emb1
  b_att    = softmax_j(max_i s)
  x2y      = sum_j b_att[j]*emb2[j]
  out      = [emb2, y2x, emb2*y2x, emb2*x2y] @ w_red + b_red

Implementation notes:
  - b_c/b_q/b_cq cancel exactly in both softmaxes (row/column constants).
  - The row softmax uses a FIXED exp shift (s - SHIFT); softmax is
    shift-invariant and the bf16/fp32 exponent range absorbs the spread.
    The row max for b_att is read straight off the s psum per slab.
  - y2x runs in fp8 (float8e4) with DoubleRow perf mode (2 k-tiles per
    matmul at 0.5 cycles/row): u is transposed raw in bf16 (so PE never
    waits on the softmax denominator), cast to fp8 as min(128*u, 240) on
    the psum drain, and the exact 1/(128 Z) row normalization is applied
    when draining the y2x psum (j back on partitions).  The fp8
    quantization noise lands only on the y2x/emb2*y2x output blocks
    (~22% of output variance) — measured rel-l2 ~7e-3 vs the 2e-2 gate.
  - Normalized y2x lets emb2 @ w1 accumulate into the same reduction
    psum as the y2x @ w2 / (emb2*y2x) @ w3 blocks (w1 block is emitted
    first: it has no in-loop deps and fills PE gaps).
  - b_att softmax also uses a fixed shift (SHIFT2): the x2y numerator
    accumulates across the j loop in psum (h<512) and a per-jt psum +
    SBUF accumulator (h>=512), removing the serial endgame chain.
  - Load phase: 6 PE transposes per 128-row chunk batched into one psum
    tile, drained with one wide strided copy; (emb2*w_cq)^T comes from a
    per-partition scale of the transposed psum; c-scores from a vector
    mul+reduce on the natural-layout staging tile.  emb1 + reduction
    weights stream on the SP HWDGE ring, emb2 on the Act ring.
"""

import numpy as np
import ml_dtypes

P = 128
XL = 2048
YL = 2048
H = 768
OUT = 300
NJT = YL // P   # 16 j tiles
NIC = XL // P   # 16 i chunks
NHC = H // P    # 6 h chunks
SLAB = 512
NSLAB = XL // SLAB  # 4
NCORES = 8
SHIFT = 10.0    # fixed exp shift for the row softmax
SHIFT2 = 12.0   # fixed exp shift for the b_att softmax (M ~ smax + c <~ 12)
U8SCALE = 128.0  # u8 = min(128*u, 240) fits fp8e4; 1/(128 Z) applied on drain

_CACHE = {}


def _fix_waits(nc, mybir, max_waits=1):
    """This walrus build rejects >1 sync wait per instruction.

    Pass 1: drop waits that are transitively implied by another wait on the
    same instruction (happens-before over per-engine / per-DMA-queue in-order
    streams plus wait edges).  Pass 2: hoist remaining extra waits onto
    same-engine NoOps inserted right before the instruction (for an in-order
    engine this blocks identically; DMA triggers are all on SP here and their
    awaited DMAs are always triggered earlier, so no cycles arise).
    """
    from collections import defaultdict

    blocks = [bb for f in nc.m.functions for bb in f.blocks]
    insts = [ins for bb in blocks for ins in bb.instructions]

    dma_types = ("InstDMACopy", "InstDmaTransposeAnt")
    eng_stream = defaultdict(list)
    queue_stream = defaultdict(list)
    sem_events = defaultdict(list)
    cum = defaultdict(int)
    for i, ins in enumerate(insts):
        eng_stream[str(ins.engine)].append(i)
        si = ins.sync_info
        if si and si.on_update:
            for u in si.on_update:
                cum[u.id] += u.update_value
                sem_events[u.id].append((cum[u.id], i))
                if type(ins).__name__ in dma_types:
                    queue_stream[u.id].append(i)

    def achiever(sem_id, val):
        for cv, i in sem_events.get(sem_id, []):
            if cv >= val:
                return i
        return None

    eng_pos, q_pos = {}, {}
    for e, lst in eng_stream.items():
        for k, i in enumerate(lst):
            eng_pos[i] = (e, k)
    for s, lst in queue_stream.items():
        for k, i in enumerate(lst):
            q_pos[i] = (s, k)

    memo = {}

    def implied(i):
        if i in memo:
            return memo[i]
        memo[i] = set()
        out = {i}
        ins = insts[i]
        if i in q_pos:
            s, k = q_pos[i]
            if k > 0:
                out |= implied(queue_stream[s][k - 1])
        e, k = eng_pos[i]
        j = k - 1
        while j >= 0:
            p = eng_stream[e][j]
            if type(insts[p]).__name__ in dma_types:
                j -= 1
                continue
            out |= implied(p)
            break
        si = ins.sync_info
        if si and si.on_wait:
            for w in si.on_wait:
                a = achiever(w.id, w.wait_value)
                if a is not None:
                    out |= implied(a)
        memo[i] = out
        return out

    # pass 1: redundancy elimination
    for i, ins in enumerate(insts):
        si = ins.sync_info
        if not (si and si.on_wait and len(si.on_wait) > max_waits):
            continue
        waits = list(si.on_wait)
        ach = [(w, achiever(w.id, w.wait_value)) for w in waits]
        keep = []
        for wi, (w, a) in enumerate(ach):
            red = False
            if a is not None:
                for wj, (w2, a2) in enumerate(ach):
                    if wi != wj and a2 is not None and a != a2 and a in implied(a2):
                        red = True
                        break
            if not red:
                keep.append(w)
        si.on_wait = keep

    # pass 2: hoist extras onto same-engine NoOps
    k = 0
    for bb in blocks:
        lst = bb.instructions
        i = 0
        while i < len(lst):
            ins = lst[i]
            si = ins.sync_info
            if si and si.on_wait and len(si.on_wait) > max_waits:
                waits = list(si.on_wait)
                extra, keep = waits[:-max_waits], waits[-max_waits:]
                si.on_wait = keep
                nops = []
                for w in extra:
                    nop = mybir.InstNoOp(name=f"I-waitfix-{k}", ins=[], outs=[])
                    k += 1
                    nop.engine = ins.engine
                    nop.sync_info = mybir.SyncInfo(on_wait=[w], on_update=[])
                    nops.append(nop)
                lst[i:i] = nops
                i += len(nops)
            i += 1


def _build():
    import concourse.bass as bass
    import concourse.tile as tile
    import concourse.mybir as mybir
    from concourse.masks import make_identity

    import math

    f32 = mybir.dt.float32
    bf16 = mybir.dt.bfloat16
    f8 = mybir.dt.float8e4
    DR = mybir.MatmulPerfMode.DoubleRow
    MUL = mybir.AluOpType.mult
    ADD = mybir.AluOpType.add
    MAX = mybir.AluOpType.max
    MIN = mybir.AluOpType.min
    EXP = mybir.ActivationFunctionType.Exp
    AXX = mybir.AxisListType.X

    nc = bass.Bass("TRN2", target_bir_lowering=False, debug=False,
                   num_devices=NCORES)

    emb1_d = nc.dram_tensor("emb1", [XL, H], bf16, kind="ExternalInput")
    emb2_d = nc.dram_tensor("emb2", [YL, H], bf16, kind="ExternalInput")
    wcb_d = nc.dram_tensor("wcb", [1, H], bf16, kind="ExternalInput")
    wq_d = nc.dram_tensor("wq", [P, NHC], bf16, kind="ExternalInput")
    wcq_d = nc.dram_tensor("wcq", [P, NHC], f32, kind="ExternalInput")
    w1_d = nc.dram_tensor("w1", [H, OUT], bf16, kind="ExternalInput")
    w2_d = nc.dram_tensor("w2", [H, OUT], bf16, kind="ExternalInput")
    w3_d = nc.dram_tensor("w3", [H, OUT], bf16, kind="ExternalInput")
    w4_d = nc.dram_tensor("w4", [H, OUT], bf16, kind="ExternalInput")
    bred_d = nc.dram_tensor("bred", [1, OUT], f32, kind="ExternalInput")
    out_d = nc.dram_tensor("out", [YL, OUT], f32, kind="ExternalOutput")

    def bcast(dram_t):
        ap = dram_t.ap()
        return bass.AP(tensor=ap.tensor, offset=ap.offset,
                       ap=[[0, P]] + list(ap.ap[1:]))

    with tile.TileContext(nc) as tc:
        with (
            tc.tile_pool(name="res", bufs=1) as res,        # resident data
            tc.tile_pool(name="stage", bufs=3) as stage,    # dma staging
            tc.tile_pool(name="small", bufs=1) as small,    # stats etc
            tc.tile_pool(name="pss", bufs=3, space="PSUM") as pss,
            tc.tile_pool(name="dpool", bufs=1, space="DRAM") as dpool,
        ):
            # ---- constants ----
            ident16 = res.tile([P, P], bf16, tag="ident16")
            make_identity(nc, ident16)
            ident32 = res.tile([P, P], f32, tag="ident32")
            make_identity(nc, ident32)
            onescol = res.tile([P, 1], bf16, tag="onescol")
            nc.vector.memset(onescol, 1.0)
            negC = res.tile([P, 1], f32, tag="negC")
            nc.vector.memset(negC, -SHIFT)

            # PE warm-up: keep the HAM activity monitor busy while the input
            # DMAs stream in.  No data deps; results are discarded.
            for wk in range(64):
                wps = pss.tile([P, P], bf16, tag="pss", name=f"warm{wk}")
                nc.tensor.transpose(wps, ident16, ident16)

            # ---- small weights first on the Act HWDGE ring (wc/wcq gate the
            # e2 load pipeline); the big w1..w4 stream after the e2 chunks ----
            wq_sb = res.tile([P, NHC], bf16, tag="wq")
            nc.scalar.dma_start(out=wq_sb, in_=wq_d[:])
            wc_bc = res.tile([P, H], bf16, tag="wc_bc")
            nc.scalar.dma_start(out=wc_bc, in_=bcast(wcb_d))
            wcq_sb = res.tile([P, NHC], f32, tag="wcq_sb")
            nc.scalar.dma_start(out=wcq_sb, in_=wcq_d[:])

            # ---- resident embeddings ----
            # e1n: natural (i_in, ic, h); e1nt: same in fp8 (y2x rhs pairs)
            # e1tt: emb1^T as (h_in, hc, i)
            # e2tt: emb2^T as (h_in, hc, j); e2ts: (emb2*w_cq)^T
            e1n = res.tile([P, NIC, H], bf16, tag="e1n")
            e1nt = res.tile([P, NIC, H], f8, tag="e1nt")
            e1tt = res.tile([P, NHC, XL], bf16, tag="e1tt")
            e2tt = res.tile([P, NHC, YL], bf16, tag="e2tt")
            e2ts = res.tile([P, NHC, YL], bf16, tag="e2ts")
            c_sb = small.tile([P, NJT], f32, tag="c_sb")

            ldp_cm = tc.tile_pool(name="ldp", bufs=3, space="PSUM")
            ldp = ldp_cm.__enter__()

            # emb1 natural in 8 wide DMAs (2 i-chunks each)
            _e1ap = emb1_d.ap()
            for a in range(8):
                nc.sync.dma_start(out=e1n[:, 2 * a:2 * (a + 1), :], in_=bass.AP(
                    tensor=_e1ap.tensor, offset=_e1ap.offset + 2 * a * P * H,
                    ap=[[H, P], [P * H, 2], [1, H]]))

            def load_e1_chunk(ic):
                isl = slice(ic * P, (ic + 1) * P)
                nc.any.tensor_copy(out=e1nt[:, ic, :], in_=e1n[:, ic, :])
                tp = ldp.tile([P, NHC, P], bf16, tag="ldp", name=f"e1tp{ic}")
                for hc in range(NHC):
                    nc.tensor.transpose(tp[:, hc, :],
                                        e1n[:, ic, hc * P:(hc + 1) * P], ident16)
                nc.any.tensor_copy(out=e1tt[:, :, isl], in_=tp)

            def load_e2_chunk(jc):
                jsl = slice(jc * P, (jc + 1) * P)
                st = stage.tile([P, H], bf16, tag="e2st", name=f"e2st{jc}")
                nc.scalar.dma_start(out=st, in_=emb2_d[jsl, :])
                # c score: per-partition dot with w_c on the natural layout
                ctmp = stage.tile([P, H], bf16, tag="ctmp", name=f"ctmp{jc}")
                nc.vector.tensor_mul(ctmp, st, wc_bc)
                nc.vector.tensor_reduce(out=c_sb[:, jc:jc + 1], in_=ctmp,
                                        axis=AXX, op=ADD)
                tpa = ldp.tile([P, NHC, P], bf16, tag="ldp", name=f"e2tpa{jc}")
                for hc in range(NHC):
                    nc.tensor.transpose(tpa[:, hc, :],
                                        st[:, hc * P:(hc + 1) * P], ident16)
                nc.any.tensor_copy(out=e2tt[:, :, jsl], in_=tpa)
                # w_cq scaling happens in the transposed layout (h on
                # partitions), straight off the transpose psum
                for hc in range(NHC):
                    nc.any.tensor_scalar_mul(e2ts[:, hc, jsl], tpa[:, hc, :],
                                             wcq_sb[:, hc:hc + 1])

            for ic in range(NIC):
                load_e1_chunk(ic)
            for jc in range(NJT):
                load_e2_chunk(jc)

            # big reduction weights stream behind the e2 chunks on the Act ring
            w1_sb = res.tile([P, NHC, OUT], bf16, tag="w1")
            w2_sb = res.tile([P, NHC, OUT], bf16, tag="w2")
            w3_sb = res.tile([P, NHC, OUT], bf16, tag="w3")
            w4_sb = res.tile([P, NHC, OUT], bf16, tag="w4")
            for w_sb, w_d in ((w1_sb, w1_d), (w2_sb, w2_d),
                              (w3_sb, w3_d), (w4_sb, w4_d)):
                ap = w_d.ap()
                nc.sync.dma_start(out=w_sb, in_=bass.AP(
                    tensor=ap.tensor, offset=ap.offset,
                    ap=[[OUT, P], [P * OUT, NHC], [1, OUT]]))
            bred_bc = res.tile([P, OUT], f32, tag="bred_bc")
            nc.sync.dma_start(out=bred_bc, in_=bcast(bred_d))

            # ---- q_row = emb1 @ w_q as fp8 value+residual (DR q-init) ----
            q8 = small.tile([1, 2, XL], f8, tag="q8")
            for sl in range(NSLAB):
                ssl = slice(sl * SLAB, (sl + 1) * SLAB)
                qp = pss.tile([1, SLAB], f32, tag="pss", name=f"qp{sl}")
                for hc in range(NHC):
                    nc.tensor.matmul(
                        qp, wq_sb[:, hc:hc + 1], e1tt[:, hc, ssl],
                        start=(hc == 0), stop=(hc == NHC - 1),
                        skip_group_check=True)
                nc.any.tensor_copy(out=q8[:, 0, ssl], in_=qp)
                nc.any.tensor_sub(q8[:, 1, ssl], qp, q8[:, 0, ssl])
            ones2 = res.tile([1, 2, P], f8, tag="ones2")
            nc.vector.memset(ones2, 1.0)

            # b_att exp bias: c - SHIFT2 (row max comes raw from the s psum)
            cbias = small.tile([P, NJT], f32, tag="cbias")
            nc.vector.tensor_scalar_add(cbias, c_sb, -SHIFT2)

            ldp_cm.__exit__(None, None, None)
            psy_cm = tc.tile_pool(name="psy", bufs=1, space="PSUM")
            psy = psy_cm.__enter__()
            pso_cm = tc.tile_pool(name="pso", bufs=2, space="PSUM")
            pso = pso_cm.__enter__()
            nmp_cm = tc.tile_pool(name="nmp", bufs=1, space="PSUM")
            nmp = nmp_cm.__enter__()

            # x2y numerator: h<512 accumulates in psum across the j loop;
            # h>=512 goes through a per-jt psum + SBUF accumulator instead
            # (saves a psum bank for the transpose pipeline).
            num1 = nmp.tile([1, SLAB], f32, tag="num1")
            x2acc2 = small.tile([1, H - SLAB], f32, tag="x2acc2")
            nc.vector.memset(x2acc2, 0.0)

            # ---- stats tiles ----
            eb_sb = small.tile([P, NJT], bf16, tag="eb")
            out_sb = res.tile([P, NJT, OUT], f32, tag="out_sb")

            # ---- main loop over j tiles ----
            sjt_cm = tc.tile_pool(name="sjt", bufs=2)
            sjt = sjt_cm.__enter__()
            for jt in range(NJT):
                jsl = slice(jt * P, (jt + 1) * P)

                # s = q + (emb2*wcq) @ emb1^T; u = exp(s - SHIFT) slab by slab.
                # Row max for b_att comes straight off the psum per slab.
                u = sjt.tile([P, XL], bf16, tag="u", name=f"u{jt}")
                Zp = sjt.tile([P, NSLAB], f32, tag="Zp", name=f"Zp{jt}")
                sm4 = sjt.tile([P, NSLAB], f32, tag="sm4", name=f"sm4{jt}")
                sps = []
                for sl in range(NSLAB):
                    ssl = slice(sl * SLAB, (sl + 1) * SLAB)
                    sp = pss.tile([P, SLAB], f32, tag="pss", name=f"sp{jt}_{sl}")
                    nc.tensor.matmul(sp, ones2, q8[:, :, ssl],
                                     perf_mode=DR, start=True, stop=False,
                                     skip_group_check=True)
                    for hc in range(NHC):
                        nc.tensor.matmul(
                            sp, e2ts[:, hc, jsl], e1tt[:, hc, ssl],
                            start=False, stop=(hc == NHC - 1),
                            skip_group_check=True)
                    nc.scalar.activation(out=u[:, ssl], in_=sp, func=EXP,
                                         bias=negC, scale=1.0,
                                         accum_out=Zp[:, sl:sl + 1])
                    sps.append(sp)
                Z = sjt.tile([P, 1], f32, tag="Z", name=f"Z{jt}")
                nc.vector.tensor_reduce(out=Z, in_=Zp, axis=AXX, op=ADD)
                rZ = sjt.tile([P, 1], f32, tag="rZ", name=f"rZ{jt}")
                nc.vector.reciprocal(out=rZ, in_=Z)
                rZ128 = sjt.tile([P, 1], f32, tag="rZ128", name=f"rZ128{jt}")
                nc.vector.tensor_scalar_mul(rZ128, rZ, 1.0 / U8SCALE)

                # transpose raw u per slab on the DMA XBAR (PE stays free and
                # never waits on Z), then one wide fp8 cast min(128*u, 240);
                # 1/(128 Z) is applied later on the y2x psum drain where j is
                # back on partitions.
                uTb = sjt.tile([P, NIC, P], bf16, tag="uTb", name=f"uTb{jt}")
                uT8 = sjt.tile([P, NIC, P], f8, tag="uT8", name=f"uT8{jt}")
                for sl in range(NSLAB):
                    ssl = slice(sl * SLAB, (sl + 1) * SLAB)
                    isl4 = slice(4 * sl, 4 * sl + 4)
                    nc.sync.dma_start(out=uTb[:, isl4, :], in_=u[:, ssl],
                                      transpose=True)
                    nc.vector.tensor_scalar(uT8[:, isl4, :], uTb[:, isl4, :],
                                            U8SCALE, 240.0, MUL, MIN)

                # y2x (natural j-major) = u8 @ emb1, fp8 DoubleRow over i pairs
                yph = psy.tile([P, SLAB], f32, tag="yph", name=f"yph{jt}")
                ypl = psy.tile([P, H - SLAB], f32, tag="ypl", name=f"ypl{jt}")
                for g in range(NIC // 2):
                    pr = slice(2 * g, 2 * g + 2)
                    nc.tensor.matmul(yph, uT8[:, pr, :], e1nt[:, pr, 0:SLAB],
                                     perf_mode=DR,
                                     start=(g == 0), stop=(g == NIC // 2 - 1),
                                     skip_group_check=True)
                for g in range(NIC // 2):
                    pr = slice(2 * g, 2 * g + 2)
                    nc.tensor.matmul(ypl, uT8[:, pr, :], e1nt[:, pr, SLAB:H],
                                     perf_mode=DR,
                                     start=(g == 0), stop=(g == NIC // 2 - 1),
                                     skip_group_check=True)

                # b_att: smax = max_i s, read off the psums late (off the
                # exp->Z->rZ critical chain in the vector queue)
                for sl in range(NSLAB):
                    nc.vector.tensor_reduce(out=sm4[:, sl:sl + 1], in_=sps[sl],
                                            axis=AXX, op=MAX)
                # drain y2x psum with the 1/(128 Z) row normalization on the
                # Act engine (keeps vector free), then transpose to (h, j)
                COPYF = mybir.ActivationFunctionType.Copy
                ynorm = sjt.tile([P, H], bf16, tag="ynorm", name=f"ynorm{jt}")
                nc.scalar.activation(out=ynorm[:, 0:SLAB], in_=yph,
                                     func=COPYF, scale=rZ128)
                nc.scalar.activation(out=ynorm[:, SLAB:H], in_=ypl,
                                     func=COPYF, scale=rZ128)
                y2xT = sjt.tile([P, NHC, P], bf16, tag="y2xT", name=f"y2xT{jt}")
                typ = pss.tile([P, NHC, P], bf16, tag="pss", name=f"typ{jt}")
                for hc in range(NHC):
                    nc.tensor.transpose(typ[:, hc, :],
                                        ynorm[:, hc * P:(hc + 1) * P], ident16)
                nc.any.tensor_copy(out=y2xT, in_=typ)
                # bl3 reads the transpose psum directly so it doesn't wait
                # for the y2xT drain (which only the w2 matmul needs)
                bl3 = sjt.tile([P, NHC, P], bf16, tag="bl3", name=f"bl3{jt}")
                for hc in range(NHC):
                    nc.vector.tensor_mul(bl3[:, hc, :], e2tt[:, hc, jsl],
                                         typ[:, hc, :])

                # reduction: [e2; y2x; e2*y2x] @ [w1; w2; w3] — the e2 block
                # has no in-loop deps, so it can fill PE gaps early
                op1 = pso.tile([P, OUT], f32, tag="pso", name=f"op1_{jt}")
                for hc in range(NHC):
                    nc.tensor.matmul(op1, e2tt[:, hc, jsl], w1_sb[:, hc, :],
                                     start=(hc == 0), stop=False,
                                     skip_group_check=True)
                for hc in range(NHC):
                    nc.tensor.matmul(op1, y2xT[:, hc, :], w2_sb[:, hc, :],
                                     start=False, stop=False,
                                     skip_group_check=True)
                for hc in range(NHC):
                    nc.tensor.matmul(op1, bl3[:, hc, :], w3_sb[:, hc, :],
                                     start=False, stop=(hc == NHC - 1),
                                     skip_group_check=True)
                nc.vector.tensor_add(out_sb[:, jt, :], op1, bred_bc)

                smax = sjt.tile([P, 1], f32, tag="smax", name=f"smax{jt}")
                nc.vector.tensor_reduce(out=smax, in_=sm4, axis=AXX, op=MAX)
                nc.scalar.activation(out=eb_sb[:, jt:jt + 1], in_=smax,
                                     func=EXP, bias=cbias[:, jt:jt + 1],
                                     scale=1.0)
                e2nj = stage.tile([P, H], bf16, tag="e2n", name=f"e2n{jt}")
                nc.sync.dma_start(out=e2nj, in_=emb2_d[jsl, :])
                nc.tensor.matmul(num1, eb_sb[:, jt:jt + 1], e2nj[:, 0:SLAB],
                                 start=(jt == 0), stop=(jt == NJT - 1),
                                 skip_group_check=True)
                np2 = pso.tile([1, H - SLAB], f32, tag="pso", name=f"np2_{jt}")
                nc.tensor.matmul(np2, eb_sb[:, jt:jt + 1], e2nj[:, SLAB:H],
                                 start=True, stop=True,
                                 skip_group_check=True)
                nc.vector.tensor_add(x2acc2, np2, x2acc2)


            sjt_cm.__exit__(None, None, None)
            post_cm = tc.tile_pool(name="post", bufs=1)
            post = post_cm.__enter__()

            # ---- x2y = NUM / DEN ----
            ebs = post.tile([P, 1], f32, tag="ebs")
            nc.vector.tensor_reduce(out=ebs, in_=eb_sb, axis=AXX, op=ADD)
            ebsb = post.tile([P, 1], bf16, tag="ebsb")
            nc.vector.tensor_copy(out=ebsb, in_=ebs)
            dps = pso.tile([1, 1], f32, tag="pso", name="dps")
            nc.tensor.matmul(dps, ebsb, onescol, start=True, stop=True,
                             skip_group_check=True)
            rden = post.tile([1, 1], f32, tag="rden")
            nc.vector.reciprocal(out=rden, in_=dps)
            x2n = post.tile([1, H], f32, tag="x2n")
            nc.any.tensor_copy(out=x2n[:, 0:SLAB], in_=num1)
            nc.any.tensor_copy(out=x2n[:, SLAB:H], in_=x2acc2)
            x2row = post.tile([1, H], f32, tag="x2row")
            nc.vector.tensor_scalar_mul(x2row, x2n, rden)
            # reshape (1, H) -> (h_in, hc) with K=1 matmuls (no DRAM bounce)
            one1 = post.tile([1, 1], f32, tag="one1")
            nc.vector.memset(one1, 1.0)
            px = pso.tile([P, NHC], f32, tag="pso", name="px")
            for hc in range(NHC):
                nc.tensor.matmul(px[:, hc:hc + 1],
                                 x2row[:, hc * P:(hc + 1) * P], one1,
                                 start=True, stop=True, skip_group_check=True)
            x2yT = post.tile([P, NHC], f32, tag="x2yT")
            nc.vector.tensor_copy(out=x2yT, in_=px)

            # R = x2y * w4
            R_sb = res.tile([P, NHC, OUT], bf16, tag="R")
            for hc in range(NHC):
                nc.vector.tensor_scalar_mul(R_sb[:, hc, :], w4_sb[:, hc, :],
                                            x2yT[:, hc:hc + 1])

            # ---- pass 2: out += emb2 @ R ----
            for jt in range(NJT):
                jsl = slice(jt * P, (jt + 1) * P)
                op2 = pso.tile([P, OUT], f32, tag="pso", name=f"op2_{jt}")
                for hc in range(NHC):
                    nc.tensor.matmul(op2, e2tt[:, hc, jsl], R_sb[:, hc, :],
                                     start=(hc == 0), stop=(hc == NHC - 1),
                                     skip_group_check=True)
                fin = stage.tile([P, OUT], f32, tag="fin", name=f"fin{jt}")
                nc.vector.tensor_add(fin, op2, out_sb[:, jt, :])
                nc.sync.dma_start(out=out_d[jsl, :], in_=fin)
            post_cm.__exit__(None, None, None)
            nmp_cm.__exit__(None, None, None)
            pso_cm.__exit__(None, None, None)
            psy_cm.__exit__(None, None, None)

    return nc


def _get_nc(drain_fix=True):
    if "nc" not in _CACHE:
        _CACHE["nc"] = _build()
    if drain_fix and not _CACHE.get("drain_fixed"):
        import concourse.mybir as mybir
        _fix_waits(_CACHE["nc"], mybir, max_waits=1)
        _CACHE["drain_fixed"] = True
    return _CACHE["nc"]


def _prep_weights(w_c, w_q, w_cq, w_red, b_red):
    bf = ml_dtypes.bfloat16
    w_red = np.asarray(w_red, dtype=np.float32)
    return {
        "wcb": np.ascontiguousarray(np.asarray(w_c, np.float32).reshape(1, H).astype(bf)),
        "wq": np.ascontiguousarray(np.asarray(w_q, np.float32).reshape(NHC, P).T.astype(bf)),
        "wcq": np.ascontiguousarray(np.asarray(w_cq, np.float32).reshape(NHC, P).T),
        "w1": np.ascontiguousarray(w_red[0:H].astype(bf)),
        "w2": np.ascontiguousarray(w_red[H:2 * H].astype(bf)),
        "w3": np.ascontiguousarray(w_red[2 * H:3 * H].astype(bf)),
        "w4": np.ascontiguousarray(w_red[3 * H:4 * H].astype(bf)),
        "bred": np.ascontiguousarray(np.asarray(b_red, np.float32).reshape(1, OUT)),
    }


def kernel(emb1, emb2, w_c, b_c, w_q, b_q, w_cq, b_cq, w_red, b_red):
    from concourse.bass_utils import run_bass_kernel_spmd

    nc = _get_nc()
    bf = ml_dtypes.bfloat16

    emb1 = np.ascontiguousarray(np.asarray(emb1, dtype=np.float32).astype(bf))
    emb2 = np.ascontiguousarray(np.asarray(emb2, dtype=np.float32).astype(bf))

    # b_c, b_q, b_cq cancel exactly in both softmaxes (per-row/col consts).
    prep = _prep_weights(w_c, w_q, w_cq, w_red, b_red)

    in_maps = []
    for b in range(NCORES):
        in_maps.append({"emb1": emb1[b], "emb2": emb2[b], **prep})
    res = run_bass_kernel_spmd(nc, in_maps, core_ids=list(range(NCORES)))
    return np.stack([res.results[i]["out"] for i in range(NCORES)], axis=0)
